# revision 2
# baseline (speedup 1.0000x reference)
"""Trainium2 Bass kernel for nn_ChebyshevKANLayer (self-contained).

Math:
    xn   = 2*(x - rowmin)/(rowmax - rowmin) - 1          per row of x [8192,1024]
    T_j  = Chebyshev polynomials of xn, j=0..8
    y    = einsum('bij,ioj->bo', T, cheby_coeffs)        [8192, 1024]

Device algorithm (data-parallel over batch, 8 NeuronCores, 1024 rows each):
    - j=0 term folded into a host-computed bias[o] = sum_i C[i,o,0].
    - P = 2*xn is computed in fp16 and PE-transposed to put the contraction
      index i on SBUF partitions.  C_1 is halved on the host so P itself is
      the j=1 matmul operand.
    - T_2..T_8 computed by the Chebyshev recurrence on the vector engine in
      fp16 (T_n = P*T_{n-1} - T_{n-2}; T_3 fused to one op).
    - y^T[o,b] accumulated in PSUM (fp32) over 64 (j,i)-chunks per output
      tile: 2048 fp16 matmuls of [128i,128o]^T x [128i,256b] per core.
    - Bias added during PSUM->SBUF copy; output written as y^T and
      transposed back on the host.
"""

import numpy as np

B, I, O, DEG = 8192, 1024, 1024, 8
NCORES = 8
BC = B // NCORES          # 1024 batch rows per core
BT = 128                  # batch tile (partitions) for stage A
NBT = BC // BT            # 8
BCW = 256                 # matmul moving-operand width (batch)
NBCH = BC // BCW          # 4
NOC = O // 128            # 8 output chunks of 128
NIB = I // 128            # 8 input-dim chunks of 128

_CACHE = {}


def _build_program():
    import concourse.bacc as bacc
    import concourse.mybir as mybir
    import concourse.tile as tile
    from contextlib import ExitStack

    f32 = mybir.dt.float32
    f16 = mybir.dt.float16
    Alu = mybir.AluOpType
    AX = mybir.AxisListType

    nc = bacc.Bacc("TRN2", target_bir_lowering=False, debug=False, num_devices=1)

    x_d = nc.dram_tensor("x_shard", [BC, I], f32, kind="ExternalInput")
    c_d = nc.dram_tensor("coeffs_t", [DEG, I, O], f16, kind="ExternalInput")
    b_d = nc.dram_tensor("bias2d", [128, NOC], f32, kind="ExternalInput")
    i_d = nc.dram_tensor("ident", [128, 128], f16, kind="ExternalInput")
    y_d = nc.dram_tensor("yT", [O, BC], f32, kind="ExternalOutput")

    with tile.TileContext(nc) as tc, ExitStack() as ctx:
        const_pool = ctx.enter_context(tc.tile_pool(name="const", bufs=1))
        cpool = ctx.enter_context(tc.tile_pool(name="cpool", bufs=1))
        ppool = ctx.enter_context(tc.tile_pool(name="ppool", bufs=1))
        xpool = ctx.enter_context(tc.tile_pool(name="xpool", bufs=2))
        spool = ctx.enter_context(tc.tile_pool(name="spool", bufs=2))
        tpool = ctx.enter_context(tc.tile_pool(name="tpool", bufs=1))
        gpool = ctx.enter_context(tc.tile_pool(name="gpool", bufs=4))
        pacc = ctx.enter_context(tc.tile_pool(name="pacc", bufs=1, space="PSUM"))
        ptr = ctx.enter_context(tc.tile_pool(name="ptr", bufs=3, space="PSUM"))

        id_sb = const_pool.tile([128, 128], f16)
        nc.sync.dma_start(id_sb[:], i_d.ap())
        bias_sb = const_pool.tile([128, NOC], f32)
        nc.sync.dma_start(bias_sb[:], b_d.ap())

        # Full coefficient tensor resident in SBUF: [i_in, j, i_blk, o] fp16
        C_sb = cpool.tile([128, DEG, NIB, O], f16)
        for j in range(DEG):
            for ib in range(NIB):
                nc.sync.dma_start(
                    C_sb[:, j, ib, :], c_d.ap()[j, ib * 128:(ib + 1) * 128, :]
                )

        # P = 2*xn, transposed: [i_in, i_blk, b] fp16
        P_buf = ppool.tile([128, NIB, BC], f16)

        def stage_a(bt):
            """Load x tile, normalize to P=2*xn (fp16), transpose into P_buf."""
            x_t = xpool.tile([128, I], f32, tag="x", name=f"x_{bt}")
            nc.sync.dma_start(x_t[:], x_d.ap()[bt * BT:(bt + 1) * BT, :])
            mx = spool.tile([128, 1], f32, tag="mx", name=f"mx_{bt}")
            nc.vector.tensor_reduce(mx[:], x_t[:], axis=AX.X, op=Alu.max)
            mn = spool.tile([128, 1], f32, tag="mn", name=f"mn_{bt}")
            nc.vector.tensor_reduce(mn[:], x_t[:], axis=AX.X, op=Alu.min)
            rng = spool.tile([128, 1], f32, tag="rng", name=f"rng_{bt}")
            nc.vector.tensor_sub(rng[:], mx[:], mn[:])
            rcp = spool.tile([128, 1], f32, tag="rcp", name=f"rcp_{bt}")
            nc.vector.reciprocal(rcp[:], rng[:])
            s2 = spool.tile([128, 1], f32, tag="s2", name=f"s2_{bt}")
            nc.vector.tensor_scalar_mul(s2[:], rcp[:], 4.0)
            # t2 = -4*min*rcp - 2
            t2a = spool.tile([128, 1], f32, tag="t2a", name=f"t2a_{bt}")
            nc.vector.scalar_tensor_tensor(
                t2a[:], mn[:], -4.0, rcp[:], op0=Alu.mult, op1=Alu.mult
            )
            t2b = spool.tile([128, 1], f32, tag="t2b", name=f"t2b_{bt}")
            nc.vector.tensor_scalar_add(t2b[:], t2a[:], -2.0)
            # P_nat = x*s2 + t2   (= 2*xn), fp16
            pn = xpool.tile([128, I], f16, tag="pn", name=f"pn_{bt}")
            nc.vector.tensor_scalar(
                pn[:], x_t[:], s2[:], t2b[:], op0=Alu.mult, op1=Alu.add
            )
            for ib in range(NIB):
                ps = ptr.tile([128, 128], f16, tag="ps", name=f"ps_{bt}_{ib}")
                nc.tensor.transpose(ps[:], pn[:, ib * 128:(ib + 1) * 128], id_sb[:])
                nc.scalar.copy(P_buf[:, ib, bt * BT:(bt + 1) * BT], ps[:])

        def emit_T(bc, n, Tp):
            """Emit DVE ops producing T_n plane [128, NIB, BCW] for chunk bc."""
            lo = bc * BCW
            Tn = tpool.tile([128, NIB, BCW], f16, tag=f"T{n}", name=f"T{n}_{bc}")
            if n >= 4 or n == 2:
                tmp = tpool.tile(
                    [128, NIB, BCW], f16, tag=f"tmp{n % 2}", name=f"tmp{n}_{bc}"
                )
            for ib in range(NIB):
                Ps = P_buf[:, ib, lo:lo + BCW]
                if n == 2:
                    # T2 = 0.5*P*P - 1
                    nc.vector.scalar_tensor_tensor(
                        tmp[:, ib, :], Ps, 0.5, Ps, op0=Alu.mult, op1=Alu.mult
                    )
                    nc.vector.tensor_scalar_add(Tn[:, ib, :], tmp[:, ib, :], -1.0)
                elif n == 3:
                    # T3 = (T2 - 0.5) * P
                    nc.vector.scalar_tensor_tensor(
                        Tn[:, ib, :], Tp[2][:, ib, :], -0.5, Ps,
                        op0=Alu.add, op1=Alu.mult,
                    )
                else:
                    nc.vector.tensor_mul(tmp[:, ib, :], Ps, Tp[n - 1][:, ib, :])
                    nc.vector.tensor_sub(
                        Tn[:, ib, :], tmp[:, ib, :], Tp[n - 2][:, ib, :]
                    )
            Tp[n] = Tn

        for bc in range(NBCH):
            # Stage A for the two b-tiles this chunk needs.
            stage_a(2 * bc)
            stage_a(2 * bc + 1)

            lo = bc * BCW
            accs = [
                pacc.tile([128, 2, BCW], f32, tag=f"acc{p}", name=f"acc{p}_{bc}")
                for p in range(4)
            ]
            Tp = {}
            for j in range(1, DEG + 1):
                if j + 1 <= DEG:
                    emit_T(bc, j + 1, Tp)
                for pair in range(4):
                    for oi in range(2):
                        oc = 2 * pair + oi
                        for ib in range(NIB):
                            mov = (
                                P_buf[:, ib, lo:lo + BCW]
                                if j == 1
                                else Tp[j][:, ib, :]
                            )
                            # One accumulation group per PSUM bank: the two
                            # oc-slices share a bank, so start/stop fire only
                            # on the bank's very first/last matmul; the
                            # per-element has_written bits make the first
                            # write to each element an overwrite.
                            nc.tensor.matmul(
                                accs[pair][:, oi, :],
                                C_sb[:, j - 1, ib, oc * 128:(oc + 1) * 128],
                                mov,
                                start=(j == 1 and ib == 0 and oi == 0),
                                stop=(j == DEG and ib == NIB - 1 and oi == 1),
                            )
            # Epilogue: bias add + store y^T chunk.
            for pair in range(4):
                for oi in range(2):
                    oc = 2 * pair + oi
                    stg = gpool.tile([128, BCW], f32, tag="stg", name=f"stg_{bc}_{oc}")
                    nc.vector.tensor_scalar_add(
                        stg[:], accs[pair][:, oi, :], bias_sb[:, oc:oc + 1]
                    )
                    nc.sync.dma_start(
                        y_d.ap()[oc * 128:(oc + 1) * 128, lo:lo + BCW], stg[:]
                    )

    nc.compile()
    return nc


def _prep_inputs(x, cheby_coeffs):
    x = np.ascontiguousarray(np.asarray(x, dtype=np.float32))
    C = np.asarray(cheby_coeffs, dtype=np.float32)
    assert x.shape == (B, I) and C.shape == (I, O, DEG + 1)

    bias = C[:, :, 0].sum(axis=0, dtype=np.float64).astype(np.float32)  # [O]
    bias2d = np.ascontiguousarray(bias.reshape(NOC, 128).T)             # [128, NOC]

    Ct = np.moveaxis(C[:, :, 1:], 2, 0).copy()                          # [DEG, I, O]
    Ct[0] *= 0.5                                                        # P = 2*xn carries j=1
    Ct16 = np.ascontiguousarray(Ct.astype(np.float16))

    ident = np.eye(128, dtype=np.float16)
    shards = x.reshape(NCORES, BC, I)
    in_maps = [
        {
            "x_shard": np.ascontiguousarray(shards[c]),
            "coeffs_t": Ct16,
            "bias2d": bias2d,
            "ident": ident,
        }
        for c in range(NCORES)
    ]
    return in_maps


def _run(in_maps, trace=False):
    from concourse import bass_utils

    if "nc" not in _CACHE:
        _CACHE["nc"] = _build_program()
    nc = _CACHE["nc"]
    res = bass_utils.run_bass_kernel_spmd(
        nc, in_maps, list(range(NCORES)), trace=trace
    )
    y = np.empty((B, O), dtype=np.float32)
    for c in range(NCORES):
        y[c * BC:(c + 1) * BC, :] = res.results[c]["yT"].T
    return y, res


def kernel(x, cheby_coeffs):
    in_maps = _prep_inputs(x, cheby_coeffs)
    y, _ = _run(in_maps, trace=False)
    return y


# revision 8
# speedup vs baseline: 1.1877x; 1.1877x over previous
"""Trainium2 Bass kernel for nn_ChebyshevKANLayer (self-contained).

Math:
    xn   = 2*(x - rowmin)/(rowmax - rowmin) - 1          per row of x [8192,1024]
    T_j  = Chebyshev polynomials of xn, j=0..8
    y    = einsum('bij,ioj->bo', T, cheby_coeffs)        [8192, 1024]

Device algorithm (data-parallel over batch, 8 NeuronCores, 1024 rows each):
    - j=0 term folded into a host-computed bias[o] = sum_i C[i,o,0].
    - P = 2*xn is computed in fp16 and PE-transposed to put the contraction
      index i on SBUF partitions.  C_1 is halved on the host so P itself is
      the j=1 matmul operand.
    - T_2..T_8 computed by the Chebyshev recurrence on the vector engine in
      fp16 (T_n = P*T_{n-1} - T_{n-2}; T_3 fused to one op).
    - y^T[o,b] accumulated in PSUM (fp32) over 64 (j,i)-chunks per output
      tile: 2048 fp16 matmuls of [128i,128o]^T x [128i,256b] per core.
    - Bias added during PSUM->SBUF copy; output written as y^T and
      transposed back on the host.
"""

import numpy as np

B, I, O, DEG = 8192, 1024, 1024, 8
NCORES = 8
BC = B // NCORES          # 1024 batch rows per core
BT = 128                  # batch tile (partitions) for stage A
NBT = BC // BT            # 8
BCW = 256                 # matmul moving-operand width (batch)
NBCH = BC // BCW          # 4
NOC = O // 128            # 8 output chunks of 128
NIB = I // 128            # 8 input-dim chunks of 128

_CACHE = {}


def _build_program():
    import concourse.bacc as bacc
    import concourse.mybir as mybir
    import concourse.tile as tile
    from contextlib import ExitStack

    f32 = mybir.dt.float32
    f16 = mybir.dt.float16
    Alu = mybir.AluOpType
    AX = mybir.AxisListType

    nc = bacc.Bacc("TRN2", target_bir_lowering=False, debug=False, num_devices=1)

    x_d = nc.dram_tensor("x_shard", [BC, I], f16, kind="ExternalInput")
    c_d = nc.dram_tensor("coeffs_t", [DEG, I, O], f16, kind="ExternalInput")
    b_d = nc.dram_tensor("bias2d", [128, NOC], f32, kind="ExternalInput")
    i_d = nc.dram_tensor("ident", [128, 128], f16, kind="ExternalInput")
    y_d = nc.dram_tensor("yT", [O, BC], f32, kind="ExternalOutput")

    with tile.TileContext(nc) as tc, ExitStack() as ctx:
        const_pool = ctx.enter_context(tc.tile_pool(name="const", bufs=1))
        cpool = ctx.enter_context(tc.tile_pool(name="cpool", bufs=1))
        ppool = ctx.enter_context(tc.tile_pool(name="ppool", bufs=1))
        xpool = ctx.enter_context(tc.tile_pool(name="xpool", bufs=2))
        spool = ctx.enter_context(tc.tile_pool(name="spool", bufs=2))
        tpool = ctx.enter_context(tc.tile_pool(name="tpool", bufs=1))
        gpool = ctx.enter_context(tc.tile_pool(name="gpool", bufs=4))
        pacc = ctx.enter_context(tc.tile_pool(name="pacc", bufs=1, space="PSUM"))
        ptr = ctx.enter_context(tc.tile_pool(name="ptr", bufs=3, space="PSUM"))

        id_sb = const_pool.tile([128, 128], f16)
        nc.sync.dma_start(id_sb[:], i_d.ap())
        bias_sb = const_pool.tile([128, NOC], f32)
        nc.sync.dma_start(bias_sb[:], b_d.ap())

        # Load all x tiles first so they hit the DMA queue heads; the 16.8 MB
        # coefficient stream follows behind them.
        x_tiles = []
        for bt in range(NBT):
            x_t = xpool.tile([128, I], f16, tag=f"x{bt}", name=f"x_{bt}", bufs=1)
            nc.sync.dma_start(x_t[:], x_d.ap()[bt * BT:(bt + 1) * BT, :])
            x_tiles.append(x_t)

        # Full coefficient tensor resident in SBUF as 64 separate tiles so
        # each matmul depends only on its own tile's DMA (a single big tile
        # would make the first matmul wait for the entire 16.8 MB load).
        C_t = [[None] * NIB for _ in range(DEG)]
        for j in range(DEG):
            for ib in range(NIB):
                ct = cpool.tile(
                    [128, O], f16, tag=f"C{j}_{ib}", name=f"C_{j}_{ib}"
                )
                nc.sync.dma_start(ct[:], c_d.ap()[j, ib * 128:(ib + 1) * 128, :])
                C_t[j][ib] = ct

        # P = 2*xn, transposed: [i_in, i_blk, b] fp16
        P_buf = ppool.tile([128, NIB, BC], f16)

        def stage_a(bt):
            """Normalize x tile to P=2*xn (fp16), transpose into P_buf."""
            x_t = x_tiles[bt]
            mx = spool.tile([128, 1], f32, tag="mx", name=f"mx_{bt}")
            nc.vector.tensor_reduce(mx[:], x_t[:], axis=AX.X, op=Alu.max)
            mn = spool.tile([128, 1], f32, tag="mn", name=f"mn_{bt}")
            nc.vector.tensor_reduce(mn[:], x_t[:], axis=AX.X, op=Alu.min)
            rng = spool.tile([128, 1], f32, tag="rng", name=f"rng_{bt}")
            nc.vector.tensor_sub(rng[:], mx[:], mn[:])
            rcp = spool.tile([128, 1], f32, tag="rcp", name=f"rcp_{bt}")
            nc.vector.reciprocal(rcp[:], rng[:])
            s2 = spool.tile([128, 1], f32, tag="s2", name=f"s2_{bt}")
            nc.vector.tensor_scalar_mul(s2[:], rcp[:], 4.0)
            # t2 = -4*min*rcp - 2
            t2a = spool.tile([128, 1], f32, tag="t2a", name=f"t2a_{bt}")
            nc.vector.scalar_tensor_tensor(
                t2a[:], mn[:], -4.0, rcp[:], op0=Alu.mult, op1=Alu.mult
            )
            t2b = spool.tile([128, 1], f32, tag="t2b", name=f"t2b_{bt}")
            nc.vector.tensor_scalar_add(t2b[:], t2a[:], -2.0)
            # P_nat = x*s2 + t2   (= 2*xn), fp16
            pn = xpool.tile([128, I], f16, tag="pn", name=f"pn_{bt}")
            nc.vector.tensor_scalar(
                pn[:], x_t[:], s2[:], t2b[:], op0=Alu.mult, op1=Alu.add
            )
            for ib in range(NIB):
                ps = ptr.tile([128, 128], f16, tag="ps", name=f"ps_{bt}_{ib}")
                nc.tensor.transpose(ps[:], pn[:, ib * 128:(ib + 1) * 128], id_sb[:])
                nc.scalar.copy(P_buf[:, ib, bt * BT:(bt + 1) * BT], ps[:])

        def emit_T(bc, n, Tp):
            """Emit DVE ops producing T_n plane [128, NIB, BCW] for chunk bc."""
            lo = bc * BCW
            Tn = tpool.tile([128, NIB, BCW], f16, tag=f"T{n}", name=f"T{n}_{bc}")
            if n >= 4 or n == 2:
                tmp = tpool.tile(
                    [128, NIB, BCW], f16, tag="tmp", name=f"tmp{n}_{bc}", bufs=1
                )
            for ib in range(NIB):
                Ps = P_buf[:, ib, lo:lo + BCW]
                if n == 2:
                    # T2 = 0.5*P*P - 1
                    nc.vector.scalar_tensor_tensor(
                        tmp[:, ib, :], Ps, 0.5, Ps, op0=Alu.mult, op1=Alu.mult
                    )
                    nc.vector.tensor_scalar_add(Tn[:, ib, :], tmp[:, ib, :], -1.0)
                elif n == 3:
                    # T3 = (T2 - 0.5) * P
                    nc.vector.scalar_tensor_tensor(
                        Tn[:, ib, :], Tp[2][:, ib, :], -0.5, Ps,
                        op0=Alu.add, op1=Alu.mult,
                    )
                else:
                    nc.vector.tensor_mul(tmp[:, ib, :], Ps, Tp[n - 1][:, ib, :])
                    nc.vector.tensor_sub(
                        Tn[:, ib, :], tmp[:, ib, :], Tp[n - 2][:, ib, :]
                    )
            Tp[n] = Tn

        for bc in range(NBCH):
            # Stage A for the two b-tiles this chunk needs.
            stage_a(2 * bc)
            stage_a(2 * bc + 1)

            lo = bc * BCW
            accs = [
                pacc.tile([128, 2, BCW], f32, tag=f"acc{p}", name=f"acc{p}_{bc}")
                for p in range(4)
            ]
            Tp = {}
            for j in range(1, DEG + 1):
                if j + 1 <= DEG:
                    emit_T(bc, j + 1, Tp)
                for pair in range(4):
                    for oi in range(2):
                        oc = 2 * pair + oi
                        for ib in range(NIB):
                            mov = (
                                P_buf[:, ib, lo:lo + BCW]
                                if j == 1
                                else Tp[j][:, ib, :]
                            )
                            # One accumulation group per PSUM bank: the two
                            # oc-slices share a bank, so start/stop fire only
                            # on the bank's very first/last matmul; the
                            # per-element has_written bits make the first
                            # write to each element an overwrite.
                            nc.tensor.matmul(
                                accs[pair][:, oi, :],
                                C_t[j - 1][ib][:, oc * 128:(oc + 1) * 128],
                                mov,
                                start=(j == 1 and ib == 0 and oi == 0),
                                stop=(j == DEG and ib == NIB - 1 and oi == 1),
                            )
            # Epilogue: bias add + store y^T chunk.
            for pair in range(4):
                for oi in range(2):
                    oc = 2 * pair + oi
                    stg = gpool.tile([128, BCW], f32, tag="stg", name=f"stg_{bc}_{oc}")
                    nc.vector.tensor_scalar_add(
                        stg[:], accs[pair][:, oi, :], bias_sb[:, oc:oc + 1]
                    )
                    nc.sync.dma_start(
                        y_d.ap()[oc * 128:(oc + 1) * 128, lo:lo + BCW], stg[:]
                    )

    nc.compile()
    return nc


def _prep_inputs(x, cheby_coeffs):
    x = np.ascontiguousarray(np.asarray(x, dtype=np.float32))
    C = np.asarray(cheby_coeffs, dtype=np.float32)
    assert x.shape == (B, I) and C.shape == (I, O, DEG + 1)

    bias = C[:, :, 0].sum(axis=0, dtype=np.float64).astype(np.float32)  # [O]
    bias2d = np.ascontiguousarray(bias.reshape(NOC, 128).T)             # [128, NOC]

    Ct = np.moveaxis(C[:, :, 1:], 2, 0).copy()                          # [DEG, I, O]
    Ct[0] *= 0.5                                                        # P = 2*xn carries j=1
    Ct16 = np.ascontiguousarray(Ct.astype(np.float16))

    ident = np.eye(128, dtype=np.float16)
    shards = x.reshape(NCORES, BC, I).astype(np.float16)
    in_maps = [
        {
            "x_shard": np.ascontiguousarray(shards[c]),
            "coeffs_t": Ct16,
            "bias2d": bias2d,
            "ident": ident,
        }
        for c in range(NCORES)
    ]
    return in_maps


def _run(in_maps, trace=False):
    from concourse import bass_utils

    if "nc" not in _CACHE:
        _CACHE["nc"] = _build_program()
    nc = _CACHE["nc"]
    res = bass_utils.run_bass_kernel_spmd(
        nc, in_maps, list(range(NCORES)), trace=trace
    )
    y = np.empty((B, O), dtype=np.float32)
    for c in range(NCORES):
        y[c * BC:(c + 1) * BC, :] = res.results[c]["yT"].T
    return y, res


def kernel(x, cheby_coeffs):
    in_maps = _prep_inputs(x, cheby_coeffs)
    y, _ = _run(in_maps, trace=False)
    return y


# revision 9
# speedup vs baseline: 1.2334x; 1.0385x over previous
"""Trainium2 Bass kernel for nn_ChebyshevKANLayer (self-contained).

Math:
    xn   = 2*(x - rowmin)/(rowmax - rowmin) - 1          per row of x [8192,1024]
    T_j  = Chebyshev polynomials of xn, j=0..8
    y    = einsum('bij,ioj->bo', T, cheby_coeffs)        [8192, 1024]

Device algorithm (data-parallel over batch, 8 NeuronCores, 1024 rows each):
    - j=0 term folded into a host-computed bias[o] = sum_i C[i,o,0], added
      during the PSUM->SBUF epilogue.
    - P = 2*xn is computed in fp16 and PE-transposed to put the contraction
      index i on SBUF partitions.  C_1 is halved on the host so P itself is
      the j=1 matmul operand.
    - T_2..T_8 computed by the Chebyshev recurrence on the vector engine in
      fp16 (T_n = P*T_{n-1} - T_{n-2}; T_3 fused to one op), pipelined one
      degree ahead of the matmul stream.
    - y[b,o] accumulated in PSUM (fp32): stationary = T_j chunk [128i,128b],
      moving = coeffs [128i,512o]; 1024 fp16 matmuls of N=512 per core,
      accumulating over all 64 (j,i)-chunks per output tile.
"""

import numpy as np

B, I, O, DEG = 8192, 1024, 1024, 8
NCORES = 8
BC = B // NCORES          # 1024 batch rows per core
BT = 128                  # batch tile (partitions) for stage A
NBT = BC // BT            # 8
BCW = 256                 # T-plane chunk width (batch)
NBCH = BC // BCW          # 4
NBS = BCW // 128          # 2 batch sub-chunks (stationary M) per chunk
OH = 512                  # matmul moving width over output dim
NOH = O // OH             # 2
NIB = I // 128            # 8 input-dim chunks of 128

_CACHE = {}


def _build_program():
    import concourse.bacc as bacc
    import concourse.mybir as mybir
    import concourse.tile as tile
    from contextlib import ExitStack

    f32 = mybir.dt.float32
    f16 = mybir.dt.float16
    Alu = mybir.AluOpType
    AX = mybir.AxisListType

    nc = bacc.Bacc("TRN2", target_bir_lowering=False, debug=False, num_devices=1)

    x_d = nc.dram_tensor("x_shard", [BC, I], f16, kind="ExternalInput")
    c_d = nc.dram_tensor("coeffs_t", [DEG, I, O], f16, kind="ExternalInput")
    b_d = nc.dram_tensor("bias_bc", [128, O], f32, kind="ExternalInput")
    i_d = nc.dram_tensor("ident", [128, 128], f16, kind="ExternalInput")
    y_d = nc.dram_tensor("y_out", [BC, O], f32, kind="ExternalOutput")

    with tile.TileContext(nc) as tc, ExitStack() as ctx:
        const_pool = ctx.enter_context(tc.tile_pool(name="const", bufs=1))
        cpool = ctx.enter_context(tc.tile_pool(name="cpool", bufs=1))
        ppool = ctx.enter_context(tc.tile_pool(name="ppool", bufs=1))
        xpool = ctx.enter_context(tc.tile_pool(name="xpool", bufs=2))
        spool = ctx.enter_context(tc.tile_pool(name="spool", bufs=2))
        tpool = ctx.enter_context(tc.tile_pool(name="tpool", bufs=1))
        gpool = ctx.enter_context(tc.tile_pool(name="gpool", bufs=2))
        pacc = ctx.enter_context(tc.tile_pool(name="pacc", bufs=1, space="PSUM"))
        ptr = ctx.enter_context(tc.tile_pool(name="ptr", bufs=3, space="PSUM"))

        id_sb = const_pool.tile([128, 128], f16)
        nc.sync.dma_start(id_sb[:], i_d.ap())
        bias_sb = const_pool.tile([128, O], f32)
        nc.sync.dma_start(bias_sb[:], b_d.ap())

        # DMA emission order: x tiles for the first chunks ahead of the
        # 16.8 MB coefficient stream, remaining x tiles interleaved between
        # early coefficient planes.
        x_tiles = [None] * NBT

        def load_x(bt):
            x_t = xpool.tile([128, I], f16, tag=f"x{bt}", name=f"x_{bt}", bufs=1)
            nc.sync.dma_start(x_t[:], x_d.ap()[bt * BT:(bt + 1) * BT, :])
            x_tiles[bt] = x_t

        # Coefficients resident in SBUF as 64 separate tiles so each matmul
        # depends only on its own tile's DMA.
        C_t = [[None] * NIB for _ in range(DEG)]

        def load_c(j):
            for ib in range(NIB):
                ct = cpool.tile(
                    [128, O], f16, tag=f"C{j}_{ib}", name=f"C_{j}_{ib}"
                )
                nc.sync.dma_start(ct[:], c_d.ap()[j, ib * 128:(ib + 1) * 128, :])
                C_t[j][ib] = ct

        load_x(0)
        load_x(1)
        load_c(0)
        load_x(2)
        load_x(3)
        load_c(1)
        load_x(4)
        load_x(5)
        load_c(2)
        load_x(6)
        load_x(7)
        for j in range(3, DEG):
            load_c(j)

        # P = 2*xn, transposed: [i_in, i_blk, b] fp16
        P_buf = ppool.tile([128, NIB, BC], f16)

        def stage_a(bt):
            """Normalize x tile to P=2*xn (fp16), transpose into P_buf."""
            x_t = x_tiles[bt]
            mx = spool.tile([128, 1], f32, tag="mx", name=f"mx_{bt}")
            nc.vector.tensor_reduce(mx[:], x_t[:], axis=AX.X, op=Alu.max)
            mn = spool.tile([128, 1], f32, tag="mn", name=f"mn_{bt}")
            nc.vector.tensor_reduce(mn[:], x_t[:], axis=AX.X, op=Alu.min)
            rng = spool.tile([128, 1], f32, tag="rng", name=f"rng_{bt}")
            nc.vector.tensor_sub(rng[:], mx[:], mn[:])
            rcp = spool.tile([128, 1], f32, tag="rcp", name=f"rcp_{bt}")
            nc.vector.reciprocal(rcp[:], rng[:])
            s2 = spool.tile([128, 1], f32, tag="s2", name=f"s2_{bt}")
            nc.vector.tensor_scalar_mul(s2[:], rcp[:], 4.0)
            # t2 = -4*min*rcp - 2
            t2a = spool.tile([128, 1], f32, tag="t2a", name=f"t2a_{bt}")
            nc.vector.scalar_tensor_tensor(
                t2a[:], mn[:], -4.0, rcp[:], op0=Alu.mult, op1=Alu.mult
            )
            t2b = spool.tile([128, 1], f32, tag="t2b", name=f"t2b_{bt}")
            nc.vector.tensor_scalar_add(t2b[:], t2a[:], -2.0)
            # P_nat = x*s2 + t2   (= 2*xn), fp16
            pn = xpool.tile([128, I], f16, tag="pn", name=f"pn_{bt}")
            nc.vector.tensor_scalar(
                pn[:], x_t[:], s2[:], t2b[:], op0=Alu.mult, op1=Alu.add
            )
            for ib in range(NIB):
                ps = ptr.tile([128, 128], f16, tag="ps", name=f"ps_{bt}_{ib}")
                nc.tensor.transpose(ps[:], pn[:, ib * 128:(ib + 1) * 128], id_sb[:])
                nc.scalar.copy(P_buf[:, ib, bt * BT:(bt + 1) * BT], ps[:])

        def emit_T(bc, n, Tp):
            """Emit DVE ops producing T_n plane [128, NIB, BCW] for chunk bc."""
            lo = bc * BCW
            Tn = tpool.tile([128, NIB, BCW], f16, tag=f"T{n}", name=f"T{n}_{bc}")
            if n >= 4 or n == 2:
                tmp = tpool.tile(
                    [128, NIB, BCW], f16, tag="tmp", name=f"tmp{n}_{bc}", bufs=1
                )
            for ib in range(NIB):
                Ps = P_buf[:, ib, lo:lo + BCW]
                if n == 2:
                    # T2 = 0.5*P*P - 1
                    nc.vector.scalar_tensor_tensor(
                        tmp[:, ib, :], Ps, 0.5, Ps, op0=Alu.mult, op1=Alu.mult
                    )
                    nc.vector.tensor_scalar_add(Tn[:, ib, :], tmp[:, ib, :], -1.0)
                elif n == 3:
                    # T3 = (T2 - 0.5) * P
                    nc.vector.scalar_tensor_tensor(
                        Tn[:, ib, :], Tp[2][:, ib, :], -0.5, Ps,
                        op0=Alu.add, op1=Alu.mult,
                    )
                else:
                    nc.vector.tensor_mul(tmp[:, ib, :], Ps, Tp[n - 1][:, ib, :])
                    nc.vector.tensor_sub(
                        Tn[:, ib, :], tmp[:, ib, :], Tp[n - 2][:, ib, :]
                    )
            Tp[n] = Tn

        for bc in range(NBCH):
            stage_a(2 * bc)
            stage_a(2 * bc + 1)

            lo = bc * BCW
            # 4 accumulators (one PSUM bank each): index 2*bs + oh
            accs = [
                pacc.tile([128, OH], f32, tag=f"acc{p}", name=f"acc{p}_{bc}")
                for p in range(NBS * NOH)
            ]
            Tp = {}
            for j in range(1, DEG + 1):
                if j + 1 <= DEG:
                    emit_T(bc, j + 1, Tp)
                for ib in range(NIB):
                    for bs in range(NBS):
                        # stationary: [128 i, 128 b] slice of T_j (P for j=1)
                        if j == 1:
                            sta = P_buf[:, ib, lo + bs * 128:lo + (bs + 1) * 128]
                        else:
                            sta = Tp[j][:, ib, bs * 128:(bs + 1) * 128]
                        for oh in range(NOH):
                            nc.tensor.matmul(
                                accs[NOH * bs + oh][:],
                                sta,
                                C_t[j - 1][ib][:, oh * OH:(oh + 1) * OH],
                                start=(j == 1 and ib == 0),
                                stop=(j == DEG and ib == NIB - 1),
                            )
            # Epilogue: bias add (fp32) + store y chunk.
            for bs in range(NBS):
                for oh in range(NOH):
                    stg = gpool.tile(
                        [128, OH], f32, tag="stg", name=f"stg_{bc}_{bs}_{oh}"
                    )
                    nc.vector.tensor_add(
                        stg[:],
                        accs[NOH * bs + oh][:],
                        bias_sb[:, oh * OH:(oh + 1) * OH],
                    )
                    nc.sync.dma_start(
                        y_d.ap()[
                            lo + bs * 128:lo + (bs + 1) * 128,
                            oh * OH:(oh + 1) * OH,
                        ],
                        stg[:],
                    )

    nc.compile()
    return nc


def _prep_inputs(x, cheby_coeffs):
    x = np.ascontiguousarray(np.asarray(x, dtype=np.float32))
    C = np.asarray(cheby_coeffs, dtype=np.float32)
    assert x.shape == (B, I) and C.shape == (I, O, DEG + 1)

    bias = C[:, :, 0].sum(axis=0, dtype=np.float64).astype(np.float32)  # [O]
    bias_bc = np.ascontiguousarray(
        np.broadcast_to(bias[None, :], (128, O))
    )

    Ct = np.moveaxis(C[:, :, 1:], 2, 0).copy()                          # [DEG, I, O]
    Ct[0] *= 0.5                                                        # P = 2*xn carries j=1
    Ct16 = np.ascontiguousarray(Ct.astype(np.float16))

    ident = np.eye(128, dtype=np.float16)
    shards = x.reshape(NCORES, BC, I).astype(np.float16)
    in_maps = [
        {
            "x_shard": np.ascontiguousarray(shards[c]),
            "coeffs_t": Ct16,
            "bias_bc": bias_bc,
            "ident": ident,
        }
        for c in range(NCORES)
    ]
    return in_maps


def _run(in_maps, trace=False):
    from concourse import bass_utils

    if "nc" not in _CACHE:
        _CACHE["nc"] = _build_program()
    nc = _CACHE["nc"]
    res = bass_utils.run_bass_kernel_spmd(
        nc, in_maps, list(range(NCORES)), trace=trace
    )
    y = np.empty((B, O), dtype=np.float32)
    for c in range(NCORES):
        y[c * BC:(c + 1) * BC, :] = res.results[c]["y_out"]
    return y, res


def kernel(x, cheby_coeffs):
    in_maps = _prep_inputs(x, cheby_coeffs)
    y, _ = _run(in_maps, trace=False)
    return y


# revision 12
# speedup vs baseline: 1.2407x; 1.0059x over previous
"""Trainium2 Bass kernel for nn_ChebyshevKANLayer (self-contained).

Math:
    xn   = 2*(x - rowmin)/(rowmax - rowmin) - 1          per row of x [8192,1024]
    T_j  = Chebyshev polynomials of xn, j=0..8
    y    = einsum('bij,ioj->bo', T, cheby_coeffs)        [8192, 1024]

Device algorithm (data-parallel over batch, 8 NeuronCores, 1024 rows each):
    - j=0 term folded into a host-computed bias[o] = sum_i C[i,o,0], added
      during the PSUM->SBUF epilogue.
    - P = 2*xn is computed in fp16 and PE-transposed to put the contraction
      index i on SBUF partitions.  C_1 is halved on the host so P itself is
      the j=1 matmul operand.
    - T_2..T_8 computed by the Chebyshev recurrence on the vector engine in
      fp16 (T_n = P*T_{n-1} - T_{n-2}; T_3 fused to one op), pipelined one
      degree ahead of the matmul stream.
    - y[b,o] accumulated in PSUM (fp32): stationary = T_j chunk [128i,128b],
      moving = coeffs [128i,512o]; 1024 fp16 matmuls of N=512 per core,
      accumulating over all 64 (j,i)-chunks per output tile.
"""

import numpy as np

B, I, O, DEG = 8192, 1024, 1024, 8
NCORES = 8
BC = B // NCORES          # 1024 batch rows per core
BT = 128                  # batch tile (partitions) for stage A
NBT = BC // BT            # 8
BCW = 256                 # T-plane chunk width (batch)
NBCH = BC // BCW          # 4
NBS = BCW // 128          # 2 batch sub-chunks (stationary M) per chunk
OH = 512                  # matmul moving width over output dim
NOH = O // OH             # 2
NIB = I // 128            # 8 input-dim chunks of 128

_CACHE = {}


def _build_program():
    import concourse.bacc as bacc
    import concourse.mybir as mybir
    import concourse.tile as tile
    from contextlib import ExitStack

    f32 = mybir.dt.float32
    f16 = mybir.dt.float16
    Alu = mybir.AluOpType
    AX = mybir.AxisListType

    nc = bacc.Bacc("TRN2", target_bir_lowering=False, debug=False, num_devices=1)

    x_d = nc.dram_tensor("x_shard", [BC, I], f16, kind="ExternalInput")
    c_d = nc.dram_tensor("coeffs_t", [DEG, I, O], f16, kind="ExternalInput")
    b_d = nc.dram_tensor("bias_bc", [128, O], f32, kind="ExternalInput")
    i_d = nc.dram_tensor("ident", [128, 128], f16, kind="ExternalInput")
    y_d = nc.dram_tensor("y_out", [BC, O], f32, kind="ExternalOutput")

    with tile.TileContext(nc) as tc, ExitStack() as ctx:
        const_pool = ctx.enter_context(tc.tile_pool(name="const", bufs=1))
        cpool = ctx.enter_context(tc.tile_pool(name="cpool", bufs=1))
        ppool = ctx.enter_context(tc.tile_pool(name="ppool", bufs=1))
        xpool = ctx.enter_context(tc.tile_pool(name="xpool", bufs=2))
        spool = ctx.enter_context(tc.tile_pool(name="spool", bufs=2))
        tpool = ctx.enter_context(tc.tile_pool(name="tpool", bufs=1))
        gpool = ctx.enter_context(tc.tile_pool(name="gpool", bufs=2))
        pacc = ctx.enter_context(tc.tile_pool(name="pacc", bufs=1, space="PSUM"))
        ptr = ctx.enter_context(tc.tile_pool(name="ptr", bufs=3, space="PSUM"))

        id_sb = const_pool.tile([128, 128], f16)
        nc.sync.dma_start(id_sb[:], i_d.ap())
        bias_sb = const_pool.tile([128, O], f32)
        nc.sync.dma_start(bias_sb[:], b_d.ap())

        # DMA emission order: x tiles for the first chunks ahead of the
        # 16.8 MB coefficient stream, remaining x tiles interleaved between
        # early coefficient planes.
        # DMA issue costs ~600ns of serial sequencer time per dma_start, so
        # spread issue across engines: x tiles from the (idle) gpsimd
        # sequencer in parallel with the coefficient stream from sync.
        x_tiles = [None] * NBT

        def load_x(bt):
            x_t = xpool.tile([128, I], f16, tag=f"x{bt}", name=f"x_{bt}", bufs=1)
            nc.gpsimd.dma_start(x_t[:], x_d.ap()[bt * BT:(bt + 1) * BT, :])
            x_tiles[bt] = x_t

        # Coefficients resident in SBUF as 16 tiles (4 i-blocks each): fine
        # enough that early matmuls only wait for their own plane, coarse
        # enough to keep DMA instruction count low.
        C_t = [[None, None] for _ in range(DEG)]

        def load_c(j):
            for h in range(2):
                ct = cpool.tile(
                    [128, 4, O], f16, tag=f"C{j}_{h}", name=f"C_{j}_{h}"
                )
                nc.sync.dma_start(
                    ct[:], c_d.ap()[j, h * 512:(h + 1) * 512, :].rearrange(
                        "(ib p) o -> p ib o", p=128
                    )
                )
                C_t[j][h] = ct

        for bt in range(NBT):
            load_x(bt)
        for j in range(DEG):
            load_c(j)

        # P = 2*xn, transposed: [i_in, i_blk, b] fp16
        P_buf = ppool.tile([128, NIB, BC], f16)

        def stage_a(bt):
            """Normalize x tile to P=2*xn (fp16), transpose into P_buf."""
            x_t = x_tiles[bt]
            mx = spool.tile([128, 1], f32, tag="mx", name=f"mx_{bt}")
            nc.vector.tensor_reduce(mx[:], x_t[:], axis=AX.X, op=Alu.max)
            mn = spool.tile([128, 1], f32, tag="mn", name=f"mn_{bt}")
            nc.vector.tensor_reduce(mn[:], x_t[:], axis=AX.X, op=Alu.min)
            rng = spool.tile([128, 1], f32, tag="rng", name=f"rng_{bt}")
            nc.vector.tensor_sub(rng[:], mx[:], mn[:])
            rcp = spool.tile([128, 1], f32, tag="rcp", name=f"rcp_{bt}")
            nc.vector.reciprocal(rcp[:], rng[:])
            s2 = spool.tile([128, 1], f32, tag="s2", name=f"s2_{bt}")
            nc.vector.tensor_scalar_mul(s2[:], rcp[:], 4.0)
            # t2 = -4*min*rcp - 2
            t2a = spool.tile([128, 1], f32, tag="t2a", name=f"t2a_{bt}")
            nc.vector.scalar_tensor_tensor(
                t2a[:], mn[:], -4.0, rcp[:], op0=Alu.mult, op1=Alu.mult
            )
            t2b = spool.tile([128, 1], f32, tag="t2b", name=f"t2b_{bt}")
            nc.vector.tensor_scalar_add(t2b[:], t2a[:], -2.0)
            # P_nat = x*s2 + t2   (= 2*xn), fp16
            pn = xpool.tile([128, I], f16, tag="pn", name=f"pn_{bt}")
            nc.vector.tensor_scalar(
                pn[:], x_t[:], s2[:], t2b[:], op0=Alu.mult, op1=Alu.add
            )
            for ib in range(NIB):
                ps = ptr.tile([128, 128], f16, tag="ps", name=f"ps_{bt}_{ib}")
                nc.tensor.transpose(ps[:], pn[:, ib * 128:(ib + 1) * 128], id_sb[:])
                nc.scalar.copy(P_buf[:, ib, bt * BT:(bt + 1) * BT], ps[:])

        def emit_T(bc, n, Tp):
            """Emit DVE ops producing T_n plane [128, NIB, BCW] for chunk bc."""
            lo = bc * BCW
            Tn = tpool.tile([128, NIB, BCW], f16, tag=f"T{n}", name=f"T{n}_{bc}")
            if n >= 4 or n == 2:
                tmp = tpool.tile(
                    [128, NIB, BCW], f16, tag="tmp", name=f"tmp{n}_{bc}", bufs=1
                )
            for ib in range(NIB):
                Ps = P_buf[:, ib, lo:lo + BCW]
                if n == 2:
                    # T2 = 0.5*P*P - 1
                    nc.vector.scalar_tensor_tensor(
                        tmp[:, ib, :], Ps, 0.5, Ps, op0=Alu.mult, op1=Alu.mult
                    )
                    nc.vector.tensor_scalar_add(Tn[:, ib, :], tmp[:, ib, :], -1.0)
                elif n == 3:
                    # T3 = (T2 - 0.5) * P
                    nc.vector.scalar_tensor_tensor(
                        Tn[:, ib, :], Tp[2][:, ib, :], -0.5, Ps,
                        op0=Alu.add, op1=Alu.mult,
                    )
                else:
                    nc.vector.tensor_mul(tmp[:, ib, :], Ps, Tp[n - 1][:, ib, :])
                    nc.vector.tensor_sub(
                        Tn[:, ib, :], tmp[:, ib, :], Tp[n - 2][:, ib, :]
                    )
            Tp[n] = Tn

        for bc in range(NBCH):
            stage_a(2 * bc)
            stage_a(2 * bc + 1)

            lo = bc * BCW
            # 4 accumulators (one PSUM bank each): index 2*bs + oh
            accs = [
                pacc.tile([128, OH], f32, tag=f"acc{p}", name=f"acc{p}_{bc}")
                for p in range(NBS * NOH)
            ]
            Tp = {}

            def mk_sta(j, ib, bs):
                # stationary: [128 i, 128 b] slice of T_j (P for j=1)
                if j == 1:
                    return P_buf[:, ib, lo + bs * 128:lo + (bs + 1) * 128]
                return Tp[j][:, ib, bs * 128:(bs + 1) * 128]

            def mk_mov(j, ib, oh):
                return C_t[j - 1][ib // 4][:, ib % 4, oh * OH:(oh + 1) * OH]

            for j in range(1, DEG + 1):
                if j + 1 <= DEG:
                    emit_T(bc, j + 1, Tp)
                if j < DEG:
                    for ib in range(NIB):
                        for bs in range(NBS):
                            sta = mk_sta(j, ib, bs)
                            for oh in range(NOH):
                                nc.tensor.matmul(
                                    accs[NOH * bs + oh][:],
                                    sta,
                                    mk_mov(j, ib, oh),
                                    start=(j == 1 and ib == 0),
                                    stop=False,
                                )
                else:
                    # Last layer: finish one PSUM bank at a time so its
                    # epilogue overlaps the other banks' matmuls.
                    for bs in range(NBS):
                        for oh in range(NOH):
                            for ib in range(NIB):
                                nc.tensor.matmul(
                                    accs[NOH * bs + oh][:],
                                    mk_sta(j, ib, bs),
                                    mk_mov(j, ib, oh),
                                    start=False,
                                    stop=(ib == NIB - 1),
                                )
            # Epilogue: bias add (fp32) + store y chunk.
            for bs in range(NBS):
                for oh in range(NOH):
                    stg = gpool.tile(
                        [128, OH], f32, tag="stg", name=f"stg_{bc}_{bs}_{oh}"
                    )
                    nc.vector.tensor_add(
                        stg[:],
                        accs[NOH * bs + oh][:],
                        bias_sb[:, oh * OH:(oh + 1) * OH],
                    )
                    nc.scalar.dma_start(
                        y_d.ap()[
                            lo + bs * 128:lo + (bs + 1) * 128,
                            oh * OH:(oh + 1) * OH,
                        ],
                        stg[:],
                    )

    nc.compile()
    return nc


def _prep_inputs(x, cheby_coeffs):
    x = np.ascontiguousarray(np.asarray(x, dtype=np.float32))
    C = np.asarray(cheby_coeffs, dtype=np.float32)
    assert x.shape == (B, I) and C.shape == (I, O, DEG + 1)

    bias = C[:, :, 0].sum(axis=0, dtype=np.float64).astype(np.float32)  # [O]
    bias_bc = np.ascontiguousarray(
        np.broadcast_to(bias[None, :], (128, O))
    )

    Ct = np.moveaxis(C[:, :, 1:], 2, 0).copy()                          # [DEG, I, O]
    Ct[0] *= 0.5                                                        # P = 2*xn carries j=1
    Ct16 = np.ascontiguousarray(Ct.astype(np.float16))

    ident = np.eye(128, dtype=np.float16)
    shards = x.reshape(NCORES, BC, I).astype(np.float16)
    in_maps = [
        {
            "x_shard": np.ascontiguousarray(shards[c]),
            "coeffs_t": Ct16,
            "bias_bc": bias_bc,
            "ident": ident,
        }
        for c in range(NCORES)
    ]
    return in_maps


def _run(in_maps, trace=False):
    from concourse import bass_utils

    if "nc" not in _CACHE:
        _CACHE["nc"] = _build_program()
    nc = _CACHE["nc"]
    res = bass_utils.run_bass_kernel_spmd(
        nc, in_maps, list(range(NCORES)), trace=trace
    )
    y = np.empty((B, O), dtype=np.float32)
    for c in range(NCORES):
        y[c * BC:(c + 1) * BC, :] = res.results[c]["y_out"]
    return y, res


def kernel(x, cheby_coeffs):
    in_maps = _prep_inputs(x, cheby_coeffs)
    y, _ = _run(in_maps, trace=False)
    return y
